# revision 31
# baseline (speedup 1.0000x reference)
"""BitStackLinear Trainium2 kernel (v2: bf16 GEMM with SBUF-resident w.T).

Computes out = x @ w.T where w = sum_i sign_i * (u_i @ vt_i), signs unpacked
from 4 packed bit-planes (one byte = 8 signs, little-endian).

Strategy: tensor-parallel over out_features across 8 NeuronCores
(1376 rows each). Per core, the o-dim is split into chunk A (4 o-tiles,
512 cols) and chunk B (7 o-tiles, 864 cols) so that reconstruction of B
overlaps the GEMM over A:

  [recon A] -> [GEMM-A over all m  ||  recon B] -> [GEMM-B over all m]

Reconstruction of w.T chunk (per 128-row k-slab, bf16, RESIDENT in SBUF):
  - DMA: vt k-slices (4 bits packed in one [16,512] tile); packed sign
    bytes broadcast 8x across partitions (4 bits side by side)
  - GpSimd: vtb4s = vtb4 * pat (folds the 2^(1-j) per-k scale, j=k%8);
    a4 = bytes4 & (1<<j) in {0, 2^j} (one i32 AND for all 4 bits)
  - PE: pr_i = vtb4s_i.T @ u_i.T -> PSUM f32 (rank-16 matmuls)
  - DVE: t_i = (a_i - 2^(j-1)) * pr_i = sign_i * r_i (STT, bf16 out)
  - PE: acc += I.T @ t_i (identity matmuls accumulate the 4 bit-planes in
    f32 PSUM; replaces 3 DVE adds)
  - ScalarE: wt[ks] = acc (evacuate to the resident bf16 w.T tile)
GEMM (all-bf16 PE, PSUM accumulation over all 32 k-slabs):
  - x.T streamed f32 per 512-col m-block, cast to bf16 (ScalarE/DVE/GpSimd)
  - stationary = resident wt[k] column tiles (bf16 -> FWL hides LDWEIGHTS)
  - ScalarE evacuation, DMA out

kernel(**inputs) takes the full unsharded inputs and returns the full output.
Host work is layout only: transposes, dtype reinterpretation, sharding.
"""

import numpy as np

import concourse.bass as bass
import concourse.bacc as bacc
import concourse.mybir as mybir
import concourse.tile as tile

W_BIT = 4
OUT_F = 11008
IN_F = 4096
RANK = 16
NCORES = 8
O_SHARD = OUT_F // NCORES          # 1376
O_TILES = (O_SHARD + 127) // 128   # 11 (last tile 96 wide)
K_TILES = IN_F // 128              # 32
MB = 512                           # m-block (x chunk width)
OT_A = 4                           # o-tiles in chunk A
O_A = OT_A * 128                   # 512
O_B = O_SHARD - O_A                # 864


def _bitstack_body(tc, aps, M):
    nc = tc.nc
    xT, qbT, uT, vt, bm, hm, pat, idn, outT = (
        aps["xT"], aps["qbT"], aps["uT"], aps["vt"], aps["bm"], aps["hm"],
        aps["pat"], aps["idn"], aps["outT"],
    )
    f32, u8, i32 = mybir.dt.float32, mybir.dt.uint8, mybir.dt.int32
    bf16 = mybir.dt.bfloat16
    n_mb = M // MB

    import contextlib
    with contextlib.ExitStack() as ctx:
        pool = ctx.enter_context(tc.tile_pool(name="sb", bufs=1))
        psum = ctx.enter_context(tc.tile_pool(name="ps", bufs=1, space="PSUM"))

        # ---- constants resident in SBUF ----
        bm_t = pool.tile([128, W_BIT * O_B], u8, name="bm_t")
        nc.sync.dma_start(bm_t, bm)
        hm_t = pool.tile([128, 1], f32, name="hm_t")
        nc.sync.dma_start(hm_t, hm)
        pat_t = pool.tile([16, 512], f32, name="pat_t")
        nc.sync.dma_start(pat_t, pat)
        idf_t = pool.tile([128, 128], f32, name="idf_t")
        nc.sync.dma_start(idf_t, idn)
        idn_t = pool.tile([128, 128], bf16, name="idn_t")
        nc.scalar.copy(idn_t, idf_t)

        # u.T resident in bf16: staged per 512-chunk through a small f32 tile
        utb = []
        for i in range(W_BIT):
            t = pool.tile([16, O_SHARD], bf16, name=f"utb{i}", tag="utb", bufs=4)
            for c0 in range(0, O_SHARD, 512):
                c1 = min(c0 + 512, O_SHARD)
                st = pool.tile([16, 512], f32, name=f"ust{i}_{c0}", tag="ost",
                               bufs=2)
                nc.sync.dma_start(st[:, :c1 - c0], uT[i, :, c0:c1])
                nc.scalar.copy(t[:, c0:c1], st[:, :c1 - c0])
            utb.append(t)

        # ---- x streaming helpers (DMA f32 chunk, cast to bf16) ----
        xbl = {}

        def emit_x(pas, mb, k, eng):
            xs = pool.tile([128, MB], f32, name=f"xs{pas}{mb}_{k}", tag="xs",
                           bufs=4)
            nc.sync.dma_start(xs, xT[k * 128:(k + 1) * 128,
                                     mb * MB:(mb + 1) * MB])
            xb = pool.tile([128, MB], bf16, name=f"xb{pas}{mb}_{k}", tag="xb",
                           bufs=60)
            if eng == "s":
                nc.scalar.copy(xb, xs)
            elif eng == "v":
                nc.vector.tensor_copy(out=xb, in_=xs)
            else:
                nc.gpsimd.tensor_copy(out=xb, in_=xs)
            xbl[(pas, mb, k)] = xb

        def prep_vt(ks, wtag, dmae):
            """vt k-slices for 4 bits + 2^(1-j) prescale (GpSimd)."""
            vtb4 = pool.tile([16, 512], f32, name=f"vtb4_{wtag}{ks}",
                             tag="vtb4", bufs=2)
            for i in range(W_BIT):
                dmae.dma_start(vtb4[:, i * 128:(i + 1) * 128],
                               vt[i, :, ks * 128:(ks + 1) * 128])
            vtb4s = pool.tile([16, 512], bf16, name=f"vtb4s_{wtag}{ks}",
                              tag="vtb4s", bufs=3)
            nc.gpsimd.tensor_tensor(out=vtb4s, in0=vtb4, in1=pat_t,
                                    op=mybir.AluOpType.mult)
            return vtb4s

        def prep_bts(ks, o0, ow, wtag, dmab):
            """Packed sign bytes broadcast 8x across partitions, 4 bits."""
            bts4 = pool.tile([128, W_BIT * ow], u8, name=f"bts{wtag}{ks}",
                             tag=f"bts{wtag}", bufs=2)
            for i in range(W_BIT):
                src = qbT[i, ks * 16:(ks + 1) * 16,
                          o0:o0 + ow][:, None, :].to_broadcast((16, 8, ow))
                dmab.dma_start(bts4[:, i * ow:(i + 1) * ow], src)
            return bts4

        def recon_start(ks, o0, ow, wtag, vtb4s, bts4):
            """AND-extract the 4 bit masks; allocate the output tile."""
            a4 = pool.tile([128, W_BIT * ow], u8, name=f"a{wtag}{ks}",
                           tag=f"a{wtag}", bufs=1 if wtag == "A" else 2)
            nc.vector.tensor_tensor(out=a4.bitcast(i32), in0=bts4.bitcast(i32),
                                    in1=bm_t.bitcast(i32)[:, 0:W_BIT * ow // 4],
                                    op=mybir.AluOpType.bitwise_and)
            st = dict(ks=ks, o0=o0, ow=ow, wtag=wtag, vtb4s=vtb4s, a4=a4,
                      tts=[], chunks=[(c0, min(c0 + 512, ow))
                                      for c0 in range(0, ow, 512)])
            st["w"] = pool.tile([128, ow], bf16, name=f"wt{wtag}{ks}",
                                tag=f"wt{wtag}", bufs=K_TILES)
            if wtag == "A":
                st["acc"] = psum.tile([128, O_A], f32, name=f"acc{wtag}{ks}",
                                      tag="acc", bufs=1)
            return st

        def recon_bit(st, i):
            """One bit-plane: pr = vt_i.T @ u_i.T, sign-apply, accumulate."""
            ks, o0, ow, wtag = st["ks"], st["o0"], st["ow"], st["wtag"]
            pr = psum.tile([128, O_B], f32, name=f"pr{wtag}{ks}_{i}",
                           tag="pr", bufs=2)
            for c0, c1 in st["chunks"]:
                nc.tensor.matmul(pr[:, c0:c1],
                                 st["vtb4s"][:, i * 128:(i + 1) * 128],
                                 utb[i][:, o0 + c0:o0 + c1],
                                 start=True, stop=True)
            t_t = pool.tile([128, O_B], bf16, name=f"t{wtag}{ks}_{i}",
                            tag="tt", bufs=2)
            nc.vector.scalar_tensor_tensor(
                out=t_t[:, :ow], in0=st["a4"][:, i * ow:(i + 1) * ow],
                scalar=hm_t, in1=pr[:, :ow],
                op0=mybir.AluOpType.subtract, op1=mybir.AluOpType.mult)
            st["tts"].append(t_t)
            w = st["w"]
            if wtag == "A":
                # bit-sum on the PE (identity matmuls): serial-A phase is
                # DVE-bound, PE has slack
                for c0, c1 in st["chunks"]:
                    nc.tensor.matmul(st["acc"][:, c0:c1], idn_t, t_t[:, c0:c1],
                                     start=(i == 0), stop=(i == W_BIT - 1))
                if i == 3:
                    nc.scalar.copy(w, st["acc"][:, :ow])
            else:
                # bit-sum on GpSimd: overlap phase is PE-bound and DVE runs
                # the STT chain; GpSimd only has the vt prescales otherwise
                if i == 1:
                    st["s01"] = pool.tile([128, O_B], bf16, name=f"s{wtag}{ks}",
                                          tag="s01", bufs=1)
                    nc.gpsimd.tensor_tensor(
                        out=st["s01"][:, :ow], in0=st["tts"][0][:, :ow],
                        in1=st["tts"][1][:, :ow], op=mybir.AluOpType.add)
                elif i == 3:
                    nc.gpsimd.tensor_tensor(
                        out=st["s01"][:, :ow], in0=st["s01"][:, :ow],
                        in1=st["tts"][2][:, :ow], op=mybir.AluOpType.add)
                    nc.gpsimd.tensor_tensor(
                        out=w, in0=st["s01"][:, :ow], in1=st["tts"][3][:, :ow],
                        op=mybir.AluOpType.add)
            return w

        def gemm_mb(pas, mb, ots, wtl, o_base):
            for ot in ots:
                ow = min(128, O_SHARD - ot * 128)
                c0 = ot * 128 - o_base
                pg = psum.tile([128, MB], f32, name=f"g{pas}{mb}_{ot}",
                               tag="pg", bufs=3)
                for k in range(K_TILES):
                    nc.tensor.matmul(
                        pg[:ow],
                        wtl[k][:, c0:c0 + ow],
                        xbl[(pas, mb, k)],
                        start=(k == 0), stop=(k == K_TILES - 1),
                    )
                ost = pool.tile([128, MB], f32, name=f"ost{pas}{mb}_{ot}",
                                tag="ost", bufs=2)
                nc.scalar.copy(ost[:ow], pg[:ow])
                nc.sync.dma_start(
                    outT[ot * 128:ot * 128 + ow, mb * MB:(mb + 1) * MB],
                    ost[:ow])

        # ---- Phase R-A: reconstruct w.T chunk A; sprinkle mb0/mb1 x prep ----
        cast_jobs = [("A", mb, k) for mb in (0, 1) for k in range(K_TILES)]
        wtA = []
        pv = prep_vt(0, "A", nc.sync)
        pb = prep_bts(0, 0, O_A, "A", nc.sync)
        for ks in range(K_TILES):
            st = recon_start(ks, 0, O_A, "A", pv, pb)
            if ks + 1 < K_TILES:
                pv = prep_vt(ks + 1, "A", nc.sync)
                pb = prep_bts(ks + 1, 0, O_A, "A", nc.sync)
            for i in range(W_BIT):
                recon_bit(st, i)
            wtA.append(st["w"])
            if cast_jobs:
                emit_x(*cast_jobs.pop(0), "s")
            if cast_jobs:
                emit_x(*cast_jobs.pop(0), "s")

        # ---- Phase G-A x R-B: GEMM chunk A overlapping recon of chunk B ----
        # Per window: sign-byte DMAs up front (scalar queue), next window's
        # vt prescale (gpsimd), then the two slabs' recon PE work sprinkled
        # between the GEMM ot-groups in bit-pairs so every pr matmul's PSUM
        # WAR and input deps resolve before the PE reaches it. Casts last
        # (they may stall on xb slot reuse and must not block the queues).
        wtB = []
        bjobs = [("B", mb, k) for mb in (0, 1) for k in range(K_TILES)]
        pvs = {s: prep_vt(s, "B", nc.gpsimd) for s in (0, 1)}
        for mb in range(n_mb):
            slabs = [s for s in (2 * mb, 2 * mb + 1) if s < K_TILES]
            sts = []
            for s in slabs:
                bts4 = prep_bts(s, O_A, O_B, "B", nc.scalar)
                sts.append(recon_start(s, O_A, O_B, "B", pvs.pop(s), bts4))
            for s in (2 * mb + 2, 2 * mb + 3):
                if s < K_TILES:
                    pvs[s] = prep_vt(s, "B", nc.gpsimd)
            for ot in range(OT_A):
                si, bp = divmod(ot, 2)
                if si < len(sts):
                    recon_bit(sts[si], 2 * bp)
                    recon_bit(sts[si], 2 * bp + 1)
                gemm_mb("A", mb, [ot], wtA, 0)
            for st in sts:
                wtB.append(st["w"])
            if mb + 2 < n_mb:
                for k in range(K_TILES):
                    emit_x("A", mb + 2, k, "s" if k % 8 < 5 else "v")
            elif mb == n_mb - 1:
                for _ in range(16):
                    if bjobs:
                        emit_x(*bjobs.pop(0), "s" if len(bjobs) % 2 else "v")

        # ---- Phase G-B: GEMM chunk B ----
        while bjobs:
            emit_x(*bjobs.pop(0), "s" if len(bjobs) % 2 else "v")
        for mb in range(n_mb):
            gemm_mb("B", mb, range(OT_A, O_TILES), wtB, O_A)
            if mb + 2 < n_mb:
                for k in range(K_TILES):
                    emit_x("B", mb + 2, k, "s" if k % 2 == 0 else "v")


def build_bass(M=8192):
    nc = bacc.Bacc("TRN2", target_bir_lowering=False, debug=False)
    f32, u8 = mybir.dt.float32, mybir.dt.uint8
    aps = {}
    aps["xT"] = nc.dram_tensor("xT", [IN_F, M], f32, kind="ExternalInput").ap()
    aps["qbT"] = nc.dram_tensor("qbT", [W_BIT, IN_F // 8, O_SHARD], u8,
                                kind="ExternalInput").ap()
    aps["uT"] = nc.dram_tensor("uT", [W_BIT, RANK, O_SHARD], f32,
                               kind="ExternalInput").ap()
    aps["vt"] = nc.dram_tensor("vt", [W_BIT, RANK, IN_F], f32,
                               kind="ExternalInput").ap()
    aps["bm"] = nc.dram_tensor("bm", [128, W_BIT * O_B], u8,
                               kind="ExternalInput").ap()
    aps["hm"] = nc.dram_tensor("hm", [128, 1], f32, kind="ExternalInput").ap()
    aps["pat"] = nc.dram_tensor("pat", [16, 512], f32, kind="ExternalInput").ap()
    aps["idn"] = nc.dram_tensor("idn", [128, 128], f32, kind="ExternalInput").ap()
    aps["outT"] = nc.dram_tensor("outT", [O_SHARD, M], f32,
                                 kind="ExternalOutput").ap()
    with tile.TileContext(nc) as tc:
        _bitstack_body(tc, aps, M)
    nc.compile()
    return nc


def prep_inputs(x, qweight, u, vt):
    """Host-side layout prep (transposes / dtype views / sharding only)."""
    M = x.shape[0] * x.shape[1]
    xT = np.ascontiguousarray(x.reshape(M, IN_F).T)
    qb = qweight.astype(np.uint8)  # values 0..255 stored in int32
    p = np.arange(128)
    bm = (np.uint8(1) << (p % 8).astype(np.uint8))[:, None] * np.ones(
        (1, W_BIT * O_B), np.uint8)
    hm = (2.0 ** ((p % 8) - 1.0)).astype(np.float32).reshape(128, 1)
    pat = np.ascontiguousarray(np.broadcast_to(
        (2.0 ** (1.0 - (np.arange(512) % 8))).astype(np.float32), (16, 512)))
    idn = np.eye(128, dtype=np.float32)
    vt_c = np.ascontiguousarray(vt)
    in_maps = []
    for c in range(NCORES):
        sl = slice(c * O_SHARD, (c + 1) * O_SHARD)
        qbT = np.ascontiguousarray(
            qb.reshape(W_BIT, OUT_F, IN_F // 8)[:, sl, :].transpose(0, 2, 1))
        uT = np.ascontiguousarray(u[:, sl, :].transpose(0, 2, 1))
        in_maps.append({
            "xT": xT, "qbT": qbT, "uT": uT, "vt": vt_c,
            "bm": bm, "hm": hm, "pat": pat, "idn": idn,
        })
    return in_maps


def _enable_ldw_opt():
    """No-op (kept for test.py compatibility). The walrus ldw-opt pass is
    incompatible with this kernel's LDWEIGHTS stream; bf16 FWL + the PE's
    background weight buffer hide the reloads instead."""
    return


def kernel(x, qweight, u, vt):
    from concourse import bass_utils
    _enable_ldw_opt()
    x = np.asarray(x)
    qweight = np.asarray(qweight)
    u = np.asarray(u)
    vt = np.asarray(vt)
    B, S, _ = x.shape
    M = B * S
    nc = build_bass(M)
    in_maps = prep_inputs(x, qweight, u, vt)
    res = bass_utils.run_bass_kernel_spmd(nc, in_maps, core_ids=list(range(NCORES)))
    out = np.empty((M, OUT_F), np.float32)
    for c in range(NCORES):
        out[:, c * O_SHARD:(c + 1) * O_SHARD] = res.results[c]["outT"].T
    return out.reshape(B, S, OUT_F)


if __name__ == "__main__":
    rng = np.random.default_rng(0)
    x = rng.standard_normal((4, 2048, IN_F)).astype(np.float32)
    qw = rng.integers(0, 256, size=(W_BIT, OUT_F * IN_F // 8)).astype(np.int32)
    uu = (rng.standard_normal((W_BIT, OUT_F, RANK)) * 0.05).astype(np.float32)
    vv = (rng.standard_normal((W_BIT, RANK, IN_F)) * 0.05).astype(np.float32)
    out = kernel(x=x, qweight=qw, u=uu, vt=vv)
    print(out.shape, out.dtype)


# revision 33
# speedup vs baseline: 1.0050x; 1.0050x over previous
"""BitStackLinear Trainium2 kernel: all-bf16 GEMM with SBUF-resident w.T.

Computes out = x @ w.T where w = sum_i sign_i * (u_i @ vt_i), signs unpacked
from 4 packed bit-planes (one byte = 8 signs, little-endian).

Strategy: tensor-parallel over out_features across 8 NeuronCores
(1376 rows each). Per core, the o-dim is split into chunk A (4 o-tiles,
512 cols) and chunk B (7 o-tiles, 864 cols) so that reconstruction of B
hides under the GEMM over A (x is streamed twice, once per chunk; DMA has
the headroom, the PE does not):

  [recon A (GEMM-A telescopes under it via per-k PSUM-group deps)]
  -> [16 windows: GEMM-A m-block || recon-B 2 k-slabs] -> [GEMM-B all m]

Reconstruction of a w.T chunk (per 128-row k-slab, bf16, RESIDENT in SBUF):
  - DMA: vt k-slices (4 bits packed in one [16,512] tile); packed sign
    bytes broadcast 8x across partitions (4 bits side by side)
  - GpSimd: vtb4s = vtb4 * pat (folds the 2^(1-j) per-k scale, j = k%8)
  - DVE: a4 = bytes4 & (1<<j) in {0, 2^j} (one i32 AND for all 4 bits)
  - PE: pr_i = vtb4s_i.T @ u_i.T -> PSUM f32 (rank-16 matmuls)
  - DVE: t_i = (a_i - 2^(j-1)) * pr_i = sign_i * r_i (STT, bf16 out)
  - bit-sum over the 4 planes: chunk A on the PE (identity-matmul PSUM
    accumulation + ScalarE evac; the serial A phase is DVE-bound), chunk B
    on DVE (the overlap phase is PE-bound)
GEMM (all-bf16 PE, PSUM accumulation over all 32 k-slabs, N=512 moving):
  - x.T streamed f32 per 512-col m-block, cast to bf16 (ScalarE/DVE), cast
    emission at window end so xb-slot reuse stalls can't block the queues
  - stationary = resident wt[k] column tiles (bf16 FWL + the PE background
    weight buffer hide LDWEIGHTS; steady-state 216 ns per 512-col matmul)
  - ScalarE evacuation, DMA out

kernel(**inputs) takes the full unsharded inputs and returns the full output.
Host work is layout only: transposes, dtype reinterpretation, sharding.
"""

import numpy as np

import concourse.bass as bass
import concourse.bacc as bacc
import concourse.mybir as mybir
import concourse.tile as tile

W_BIT = 4
OUT_F = 11008
IN_F = 4096
RANK = 16
NCORES = 8
O_SHARD = OUT_F // NCORES          # 1376
O_TILES = (O_SHARD + 127) // 128   # 11 (last tile 96 wide)
K_TILES = IN_F // 128              # 32
MB = 512                           # m-block (x chunk width)
OT_A = 4                           # o-tiles in chunk A
O_A = OT_A * 128                   # 512
O_B = O_SHARD - O_A                # 864


def _bitstack_body(tc, aps, M):
    nc = tc.nc
    xT, qbT, uT, vt, bm, hm, pat, idn, outT = (
        aps["xT"], aps["qbT"], aps["uT"], aps["vt"], aps["bm"], aps["hm"],
        aps["pat"], aps["idn"], aps["outT"],
    )
    f32, u8, i32 = mybir.dt.float32, mybir.dt.uint8, mybir.dt.int32
    bf16 = mybir.dt.bfloat16
    n_mb = M // MB

    import contextlib
    with contextlib.ExitStack() as ctx:
        pool = ctx.enter_context(tc.tile_pool(name="sb", bufs=1))
        psum = ctx.enter_context(tc.tile_pool(name="ps", bufs=1, space="PSUM"))

        # ---- constants resident in SBUF ----
        bm_t = pool.tile([128, W_BIT * O_B], u8, name="bm_t")
        nc.sync.dma_start(bm_t, bm)
        hm_t = pool.tile([128, 1], f32, name="hm_t")
        nc.sync.dma_start(hm_t, hm)
        pat_t = pool.tile([16, 512], f32, name="pat_t")
        nc.sync.dma_start(pat_t, pat)
        idf_t = pool.tile([128, 128], f32, name="idf_t")
        nc.sync.dma_start(idf_t, idn)
        idn_t = pool.tile([128, 128], bf16, name="idn_t")
        nc.scalar.copy(idn_t, idf_t)

        # u.T resident in bf16: staged per 512-chunk through a small f32 tile
        utb = []
        for i in range(W_BIT):
            t = pool.tile([16, O_SHARD], bf16, name=f"utb{i}", tag="utb", bufs=4)
            for c0 in range(0, O_SHARD, 512):
                c1 = min(c0 + 512, O_SHARD)
                st = pool.tile([16, 512], f32, name=f"ust{i}_{c0}", tag="ost",
                               bufs=2)
                nc.sync.dma_start(st[:, :c1 - c0], uT[i, :, c0:c1])
                nc.scalar.copy(t[:, c0:c1], st[:, :c1 - c0])
            utb.append(t)

        # ---- x streaming helpers (DMA f32 chunk, cast to bf16) ----
        xbl = {}

        def emit_x(pas, mb, k, eng):
            xs = pool.tile([128, MB], f32, name=f"xs{pas}{mb}_{k}", tag="xs",
                           bufs=4)
            nc.sync.dma_start(xs, xT[k * 128:(k + 1) * 128,
                                     mb * MB:(mb + 1) * MB])
            xb = pool.tile([128, MB], bf16, name=f"xb{pas}{mb}_{k}", tag="xb",
                           bufs=60)
            if eng == "s":
                nc.scalar.copy(xb, xs)
            elif eng == "v":
                nc.vector.tensor_copy(out=xb, in_=xs)
            else:
                nc.gpsimd.tensor_copy(out=xb, in_=xs)
            xbl[(pas, mb, k)] = xb

        def prep_vt(ks, wtag, dmae):
            """vt k-slices for 4 bits + 2^(1-j) prescale (GpSimd)."""
            vtb4 = pool.tile([16, 512], f32, name=f"vtb4_{wtag}{ks}",
                             tag="vtb4", bufs=2)
            for i in range(W_BIT):
                dmae.dma_start(vtb4[:, i * 128:(i + 1) * 128],
                               vt[i, :, ks * 128:(ks + 1) * 128])
            vtb4s = pool.tile([16, 512], bf16, name=f"vtb4s_{wtag}{ks}",
                              tag="vtb4s", bufs=3)
            nc.gpsimd.tensor_tensor(out=vtb4s, in0=vtb4, in1=pat_t,
                                    op=mybir.AluOpType.mult)
            return vtb4s

        def prep_bts(ks, o0, ow, wtag, dmab):
            """Packed sign bytes broadcast 8x across partitions, 4 bits."""
            bts4 = pool.tile([128, W_BIT * ow], u8, name=f"bts{wtag}{ks}",
                             tag=f"bts{wtag}", bufs=2)
            for i in range(W_BIT):
                src = qbT[i, ks * 16:(ks + 1) * 16,
                          o0:o0 + ow][:, None, :].to_broadcast((16, 8, ow))
                dmab.dma_start(bts4[:, i * ow:(i + 1) * ow], src)
            return bts4

        def recon_start(ks, o0, ow, wtag, vtb4s, bts4):
            """AND-extract the 4 bit masks; allocate the output tile."""
            a4 = pool.tile([128, W_BIT * ow], u8, name=f"a{wtag}{ks}",
                           tag=f"a{wtag}", bufs=1 if wtag == "A" else 2)
            nc.vector.tensor_tensor(out=a4.bitcast(i32), in0=bts4.bitcast(i32),
                                    in1=bm_t.bitcast(i32)[:, 0:W_BIT * ow // 4],
                                    op=mybir.AluOpType.bitwise_and)
            st = dict(ks=ks, o0=o0, ow=ow, wtag=wtag, vtb4s=vtb4s, a4=a4,
                      tts=[], chunks=[(c0, min(c0 + 512, ow))
                                      for c0 in range(0, ow, 512)])
            st["w"] = pool.tile([128, ow], bf16, name=f"wt{wtag}{ks}",
                                tag=f"wt{wtag}", bufs=K_TILES)
            if wtag == "A":
                st["acc"] = psum.tile([128, O_A], f32, name=f"acc{wtag}{ks}",
                                      tag="acc", bufs=1)
            return st

        def recon_bit(st, i):
            """One bit-plane: pr = vt_i.T @ u_i.T, sign-apply, accumulate."""
            ks, o0, ow, wtag = st["ks"], st["o0"], st["ow"], st["wtag"]
            pr = psum.tile([128, O_B], f32, name=f"pr{wtag}{ks}_{i}",
                           tag="pr", bufs=2)
            for c0, c1 in st["chunks"]:
                nc.tensor.matmul(pr[:, c0:c1],
                                 st["vtb4s"][:, i * 128:(i + 1) * 128],
                                 utb[i][:, o0 + c0:o0 + c1],
                                 start=True, stop=True)
            t_t = pool.tile([128, O_B], bf16, name=f"t{wtag}{ks}_{i}",
                            tag="tt", bufs=2)
            nc.vector.scalar_tensor_tensor(
                out=t_t[:, :ow], in0=st["a4"][:, i * ow:(i + 1) * ow],
                scalar=hm_t, in1=pr[:, :ow],
                op0=mybir.AluOpType.subtract, op1=mybir.AluOpType.mult)
            st["tts"].append(t_t)
            w = st["w"]
            if wtag == "A":
                # bit-sum on the PE (identity matmuls): serial-A phase is
                # DVE-bound, PE has slack
                for c0, c1 in st["chunks"]:
                    nc.tensor.matmul(st["acc"][:, c0:c1], idn_t, t_t[:, c0:c1],
                                     start=(i == 0), stop=(i == W_BIT - 1))
                if i == 3:
                    nc.scalar.copy(w, st["acc"][:, :ow])
            else:
                # bit-sum on DVE: overlap phase is PE-bound, DVE has slack
                if i == 1:
                    st["s01"] = pool.tile([128, O_B], bf16, name=f"s{wtag}{ks}",
                                          tag="s01", bufs=1)
                    nc.vector.tensor_tensor(
                        out=st["s01"][:, :ow], in0=st["tts"][0][:, :ow],
                        in1=st["tts"][1][:, :ow], op=mybir.AluOpType.add)
                elif i == 3:
                    nc.vector.tensor_tensor(
                        out=st["s01"][:, :ow], in0=st["s01"][:, :ow],
                        in1=st["tts"][2][:, :ow], op=mybir.AluOpType.add)
                    nc.vector.tensor_tensor(
                        out=w, in0=st["s01"][:, :ow], in1=st["tts"][3][:, :ow],
                        op=mybir.AluOpType.add)
            return w

        def gemm_mb(pas, mb, ots, wtl, o_base):
            for ot in ots:
                ow = min(128, O_SHARD - ot * 128)
                c0 = ot * 128 - o_base
                pg = psum.tile([128, MB], f32, name=f"g{pas}{mb}_{ot}",
                               tag="pg", bufs=3)
                for k in range(K_TILES):
                    nc.tensor.matmul(
                        pg[:ow],
                        wtl[k][:, c0:c0 + ow],
                        xbl[(pas, mb, k)],
                        start=(k == 0), stop=(k == K_TILES - 1),
                    )
                ost = pool.tile([128, MB], f32, name=f"ost{pas}{mb}_{ot}",
                                tag="ost", bufs=2)
                nc.scalar.copy(ost[:ow], pg[:ow])
                nc.sync.dma_start(
                    outT[ot * 128:ot * 128 + ow, mb * MB:(mb + 1) * MB],
                    ost[:ow])

        # ---- Phase R-A: reconstruct w.T chunk A; sprinkle mb0/mb1 x prep ----
        cast_jobs = [("A", mb, k) for mb in (0, 1) for k in range(K_TILES)]
        wtA = []
        pv = prep_vt(0, "A", nc.sync)
        pb = prep_bts(0, 0, O_A, "A", nc.sync)
        for ks in range(K_TILES):
            st = recon_start(ks, 0, O_A, "A", pv, pb)
            if ks + 1 < K_TILES:
                pv = prep_vt(ks + 1, "A", nc.sync)
                pb = prep_bts(ks + 1, 0, O_A, "A", nc.sync)
            for i in range(W_BIT):
                recon_bit(st, i)
            wtA.append(st["w"])
            if cast_jobs:
                emit_x(*cast_jobs.pop(0), "s")
            if cast_jobs:
                emit_x(*cast_jobs.pop(0), "s")

        # ---- Phase G-A x R-B: GEMM chunk A overlapping recon of chunk B ----
        # Per window: sign-byte DMAs up front (scalar queue), next window's
        # vt prescale (gpsimd), then the two slabs' recon PE work sprinkled
        # between the GEMM ot-groups in bit-pairs so every pr matmul's PSUM
        # WAR and input deps resolve before the PE reaches it. Casts last
        # (they may stall on xb slot reuse and must not block the queues).
        wtB = []
        bjobs = [("B", mb, k) for mb in (0, 1) for k in range(K_TILES)]
        pvs = {s: prep_vt(s, "B", nc.gpsimd) for s in (0, 1)}
        for mb in range(n_mb):
            slabs = [s for s in (2 * mb, 2 * mb + 1) if s < K_TILES]
            sts = []
            for s in slabs:
                bts4 = prep_bts(s, O_A, O_B, "B", nc.scalar)
                sts.append(recon_start(s, O_A, O_B, "B", pvs.pop(s), bts4))
            for s in (2 * mb + 2, 2 * mb + 3):
                if s < K_TILES:
                    pvs[s] = prep_vt(s, "B", nc.gpsimd)
            gemm_mb("A", mb, range(OT_A), wtA, 0)
            for st in sts:
                for i in range(W_BIT):
                    recon_bit(st, i)
                wtB.append(st["w"])
            if mb + 2 < n_mb:
                for k in range(K_TILES):
                    emit_x("A", mb + 2, k, "s" if k % 8 < 5 else "v")
            elif mb == n_mb - 1:
                for _ in range(16):
                    if bjobs:
                        emit_x(*bjobs.pop(0), "s" if len(bjobs) % 2 else "v")

        # ---- Phase G-B: GEMM chunk B ----
        while bjobs:
            emit_x(*bjobs.pop(0), "s" if len(bjobs) % 2 else "v")
        for mb in range(n_mb):
            gemm_mb("B", mb, range(OT_A, O_TILES), wtB, O_A)
            if mb + 2 < n_mb:
                for k in range(K_TILES):
                    emit_x("B", mb + 2, k, "s" if k % 2 == 0 else "v")


def build_bass(M=8192):
    nc = bacc.Bacc("TRN2", target_bir_lowering=False, debug=False)
    f32, u8 = mybir.dt.float32, mybir.dt.uint8
    aps = {}
    aps["xT"] = nc.dram_tensor("xT", [IN_F, M], f32, kind="ExternalInput").ap()
    aps["qbT"] = nc.dram_tensor("qbT", [W_BIT, IN_F // 8, O_SHARD], u8,
                                kind="ExternalInput").ap()
    aps["uT"] = nc.dram_tensor("uT", [W_BIT, RANK, O_SHARD], f32,
                               kind="ExternalInput").ap()
    aps["vt"] = nc.dram_tensor("vt", [W_BIT, RANK, IN_F], f32,
                               kind="ExternalInput").ap()
    aps["bm"] = nc.dram_tensor("bm", [128, W_BIT * O_B], u8,
                               kind="ExternalInput").ap()
    aps["hm"] = nc.dram_tensor("hm", [128, 1], f32, kind="ExternalInput").ap()
    aps["pat"] = nc.dram_tensor("pat", [16, 512], f32, kind="ExternalInput").ap()
    aps["idn"] = nc.dram_tensor("idn", [128, 128], f32, kind="ExternalInput").ap()
    aps["outT"] = nc.dram_tensor("outT", [O_SHARD, M], f32,
                                 kind="ExternalOutput").ap()
    with tile.TileContext(nc) as tc:
        _bitstack_body(tc, aps, M)
    nc.compile()
    return nc


def prep_inputs(x, qweight, u, vt):
    """Host-side layout prep (transposes / dtype views / sharding only)."""
    M = x.shape[0] * x.shape[1]
    xT = np.ascontiguousarray(x.reshape(M, IN_F).T)
    qb = qweight.astype(np.uint8)  # values 0..255 stored in int32
    p = np.arange(128)
    bm = (np.uint8(1) << (p % 8).astype(np.uint8))[:, None] * np.ones(
        (1, W_BIT * O_B), np.uint8)
    hm = (2.0 ** ((p % 8) - 1.0)).astype(np.float32).reshape(128, 1)
    pat = np.ascontiguousarray(np.broadcast_to(
        (2.0 ** (1.0 - (np.arange(512) % 8))).astype(np.float32), (16, 512)))
    idn = np.eye(128, dtype=np.float32)
    vt_c = np.ascontiguousarray(vt)
    in_maps = []
    for c in range(NCORES):
        sl = slice(c * O_SHARD, (c + 1) * O_SHARD)
        qbT = np.ascontiguousarray(
            qb.reshape(W_BIT, OUT_F, IN_F // 8)[:, sl, :].transpose(0, 2, 1))
        uT = np.ascontiguousarray(u[:, sl, :].transpose(0, 2, 1))
        in_maps.append({
            "xT": xT, "qbT": qbT, "uT": uT, "vt": vt_c,
            "bm": bm, "hm": hm, "pat": pat, "idn": idn,
        })
    return in_maps


def _enable_ldw_opt():
    """No-op (kept for test.py compatibility). The walrus ldw-opt pass is
    incompatible with this kernel's LDWEIGHTS stream; bf16 FWL + the PE's
    background weight buffer hide the reloads instead."""
    return


def kernel(x, qweight, u, vt):
    from concourse import bass_utils
    _enable_ldw_opt()
    x = np.asarray(x)
    qweight = np.asarray(qweight)
    u = np.asarray(u)
    vt = np.asarray(vt)
    B, S, _ = x.shape
    M = B * S
    nc = build_bass(M)
    in_maps = prep_inputs(x, qweight, u, vt)
    res = bass_utils.run_bass_kernel_spmd(nc, in_maps, core_ids=list(range(NCORES)))
    out = np.empty((M, OUT_F), np.float32)
    for c in range(NCORES):
        out[:, c * O_SHARD:(c + 1) * O_SHARD] = res.results[c]["outT"].T
    return out.reshape(B, S, OUT_F)


if __name__ == "__main__":
    rng = np.random.default_rng(0)
    x = rng.standard_normal((4, 2048, IN_F)).astype(np.float32)
    qw = rng.integers(0, 256, size=(W_BIT, OUT_F * IN_F // 8)).astype(np.int32)
    uu = (rng.standard_normal((W_BIT, OUT_F, RANK)) * 0.05).astype(np.float32)
    vv = (rng.standard_normal((W_BIT, RANK, IN_F)) * 0.05).astype(np.float32)
    out = kernel(x=x, qweight=qw, u=uu, vt=vv)
    print(out.shape, out.dtype)
